# revision 1
# baseline (speedup 1.0000x reference)
"""Trainium2 Bass kernel for CustomMinkowskiLayerNorm.

Math (matches the jax reference):
    counts[b]  = #points with batch_indices == b           (clamped >= 1)
    mean[b,c]  = sum_{i in b} x[i,c] / counts[b]
    var[b,c]   = sum_{i in b} (x[i,c]-mean)^2 / counts[b]  (= E[x^2]-mean^2)
    out[i,c]   = (x[i,c]-mean[b_i,c]) / sqrt(var[b_i,c]+eps) * gamma[c] + beta[c]

Sharding: batch_indices is sorted, BATCH == n_cores == 8, so each core owns
exactly one batch segment -> all segment reductions are core-local, no
collectives. The host splits at segment boundaries (searchsorted), transposes
each segment to channel-major layout and zero-pads to a fixed shape:

    xt[p, f], p in [0,128): partition p < 64  = channel p,  points [0, 65536)
                            partition p >= 64 = channel p-64, points [65536, 131072)

Channel-major layout makes the per-channel segment reduction a free-dim
reduction (bn_stats) and the normalization a single per-partition
tensor_scalar (x*s + t) that runs in the DVE 2x fp32 perf mode.

Device program (per core, identical SPMD):
  pass 1: DMA 32 tiles of [128, 2048]; 4x bn_stats per tile into a stats
          buffer; first NCACHE tiles stay resident in SBUF.
  stats:  bn_aggr -> (mean, var) of each padded half-row; convert to raw
          (sum, sumsq); fold partitions p/p+64 (SBUF->SBUF DMA shift); apply
          1/count from the host; rstd = 1/sqrt(var+eps) with 2 Newton
          iterations (ACT sqrt table is low precision); s = gamma*rstd,
          t = beta - mean*s, replicated to both partition halves.
  pass 2: out_tile = x_tile * s + t (tensor_scalar, in-place for cached
          tiles); DMA back. Only non-cached tiles are re-read from HBM.
"""

import os
import sys

for _p in ("/opt/trn_rl_repo", "/root/.axon_site/_ro/trn_rl_repo"):
    if os.path.isdir(_p) and _p not in sys.path:
        sys.path.append(_p)

from contextlib import ExitStack

import numpy as np

import concourse.bacc as bacc
import concourse.tile as tile
from concourse import mybir
from concourse._compat import with_exitstack
from concourse.bass_utils import run_bass_kernel_spmd

F32 = mybir.dt.float32

N = 1_000_000
C = 64
BATCH = 8
EPS = 1e-5

P = 128            # SBUF partitions
F_HALF = 65536     # free length per partition (points per half)
PAD = 2 * F_HALF   # max points per segment (max observed ~126.5k)
F_TILE = 2048      # free elems per tile -> [128, 2048] f32 = 1 MiB per DMA
NT = F_HALF // F_TILE          # 32 tiles
BN_F = 512                     # bn_stats free-dim max
BN_PER_TILE = F_TILE // BN_F   # 4
NCACHE = 16                    # tiles kept resident in SBUF between passes

_mult = mybir.AluOpType.mult
_add = mybir.AluOpType.add


@with_exitstack
def _body(ctx: ExitStack, tc: tile.TileContext, out_ap, xt_ap, invn_ap, gcol_ap, bcol_ap):
    nc = tc.nc

    cache = ctx.enter_context(tc.tile_pool(name="cache", bufs=NCACHE))
    lpool = ctx.enter_context(tc.tile_pool(name="lpool", bufs=3))
    p2pool = ctx.enter_context(tc.tile_pool(name="p2pool", bufs=4))
    small = ctx.enter_context(tc.tile_pool(name="small", bufs=1))

    # small per-partition inputs
    invn_sb = small.tile([P, 1], F32, tag="invn")
    gcol_sb = small.tile([P, 1], F32, tag="gcol")
    bcol_sb = small.tile([P, 1], F32, tag="bcol")
    nc.sync.dma_start(out=invn_sb, in_=invn_ap)
    nc.sync.dma_start(out=gcol_sb, in_=gcol_ap)
    nc.sync.dma_start(out=bcol_sb, in_=bcol_ap)

    stats = small.tile([P, NT * BN_PER_TILE, 6], F32, tag="stats")

    # ---- pass 1: stream all tiles, bn_stats each 512-chunk ----
    cached = []
    for t in range(NT):
        sl = slice(t * F_TILE, (t + 1) * F_TILE)
        if t < NCACHE:
            xt = cache.tile([P, F_TILE], F32, tag="c")
            cached.append(xt)
        else:
            xt = lpool.tile([P, F_TILE], F32, tag="l")
        nc.sync.dma_start(out=xt, in_=xt_ap[:, sl])
        for j in range(BN_PER_TILE):
            nc.vector.bn_stats(
                out=stats[:, t * BN_PER_TILE + j, :],
                in_=xt[:, j * BN_F : (j + 1) * BN_F],
            )

    # ---- aggregate stats ----
    mv = small.tile([P, 2], F32, tag="mv")          # mean, var over padded row
    nc.vector.bn_aggr(out=mv, in_=stats)

    sums = small.tile([P, 2], F32, tag="sums")      # raw (sum, sumsq)
    # sum = mean * F_HALF
    nc.vector.tensor_scalar_mul(out=sums[:, 0:1], in0=mv[:, 0:1], scalar1=float(F_HALF))
    # sumsq = (var + mean^2) * F_HALF
    msq = small.tile([P, 1], F32, tag="msq")
    nc.vector.tensor_mul(out=msq, in0=mv[:, 0:1], in1=mv[:, 0:1])
    nc.vector.tensor_add(out=msq, in0=msq, in1=mv[:, 1:2])
    nc.vector.tensor_scalar_mul(out=sums[:, 1:2], in0=msq, scalar1=float(F_HALF))

    # ---- fold the two point-halves: tot[c] = sums[c] + sums[c+64] ----
    shift = small.tile([P, 2], F32, tag="shift")
    nc.sync.dma_start(out=shift[0:64, :], in_=sums[64:128, :])
    tot = small.tile([P, 2], F32, tag="tot")
    nc.vector.tensor_add(out=tot[0:64, :], in0=sums[0:64, :], in1=shift[0:64, :])
    # replicate totals to the upper partition half
    nc.sync.dma_start(out=tot[64:128, :], in_=tot[0:64, :])

    # ---- per-channel coefficients ----
    mean = small.tile([P, 1], F32, tag="mean")
    nc.vector.tensor_scalar(out=mean, in0=tot[:, 0:1], scalar1=invn_sb[:, 0:1],
                            scalar2=None, op0=_mult)
    var = small.tile([P, 1], F32, tag="var")
    nc.vector.tensor_scalar(out=var, in0=tot[:, 1:2], scalar1=invn_sb[:, 0:1],
                            scalar2=None, op0=_mult)
    m2 = small.tile([P, 1], F32, tag="m2")
    nc.vector.tensor_mul(out=m2, in0=mean, in1=mean)
    nc.vector.tensor_sub(out=var, in0=var, in1=m2)
    # v = max(var, 0) + eps
    v = small.tile([P, 1], F32, tag="v")
    nc.vector.tensor_scalar(out=v, in0=var, scalar1=0.0, scalar2=EPS,
                            op0=mybir.AluOpType.max, op1=_add)
    # r ~= 1/sqrt(v): ACT sqrt (low precision) + reciprocal, then 2 Newton steps
    r = small.tile([P, 1], F32, tag="r")
    nc.scalar.activation(out=r, in_=v, func=mybir.ActivationFunctionType.Sqrt)
    nc.vector.reciprocal(out=r, in_=r)
    a = small.tile([P, 1], F32, tag="a")
    for _ in range(2):
        nc.vector.tensor_mul(out=a, in0=r, in1=r)
        nc.vector.tensor_mul(out=a, in0=a, in1=v)
        nc.vector.tensor_scalar(out=a, in0=a, scalar1=-0.5, scalar2=1.5,
                                op0=_mult, op1=_add)
        nc.vector.tensor_mul(out=r, in0=r, in1=a)
    s_col = small.tile([P, 1], F32, tag="s_col")
    nc.vector.tensor_mul(out=s_col, in0=r, in1=gcol_sb)
    t_col = small.tile([P, 1], F32, tag="t_col")
    nc.vector.tensor_mul(out=t_col, in0=mean, in1=s_col)
    nc.vector.tensor_sub(out=t_col, in0=bcol_sb, in1=t_col)

    # ---- pass 2: out = x*s + t ----
    # non-cached tiles first so their loads can overlap pass-1 tail
    order = list(range(NCACHE, NT)) + list(range(NCACHE))
    for t in order:
        sl = slice(t * F_TILE, (t + 1) * F_TILE)
        if t < NCACHE:
            xt = cached[t]
        else:
            xt = p2pool.tile([P, F_TILE], F32, tag="p2")
            nc.sync.dma_start(out=xt, in_=xt_ap[:, sl])
        nc.vector.tensor_scalar(out=xt, in0=xt, scalar1=s_col[:, 0:1],
                                scalar2=t_col[:, 0:1], op0=_mult, op1=_add)
        nc.scalar.dma_start(out=out_ap[:, sl], in_=xt)


_NC_CACHE = {}


def _build_program():
    if "nc" in _NC_CACHE:
        return _NC_CACHE["nc"]
    nc = bacc.Bacc("TRN2", target_bir_lowering=False, debug=False, num_devices=BATCH)
    xt = nc.dram_tensor("xt", [P, F_HALF], F32, kind="ExternalInput").ap()
    invn = nc.dram_tensor("invn", [P, 1], F32, kind="ExternalInput").ap()
    gcol = nc.dram_tensor("gcol", [P, 1], F32, kind="ExternalInput").ap()
    bcol = nc.dram_tensor("bcol", [P, 1], F32, kind="ExternalInput").ap()
    out = nc.dram_tensor("out", [P, F_HALF], F32, kind="ExternalOutput").ap()
    with tile.TileContext(nc) as tc:
        _body(tc, out, xt, invn, gcol, bcol)
    nc.compile()
    _NC_CACHE["nc"] = nc
    return nc


def _prepare(features, batch_indices, gamma, beta):
    features = np.asarray(features, dtype=np.float32)
    batch_indices = np.asarray(batch_indices, dtype=np.int32)
    gamma = np.asarray(gamma, dtype=np.float32)
    beta = np.asarray(beta, dtype=np.float32)

    bounds = np.searchsorted(batch_indices, np.arange(BATCH + 1), side="left")
    gcol = np.concatenate([gamma, gamma]).reshape(P, 1).astype(np.float32)
    bcol = np.concatenate([beta, beta]).reshape(P, 1).astype(np.float32)

    in_maps = []
    for b in range(BATCH):
        s, e = int(bounds[b]), int(bounds[b + 1])
        cnt = e - s
        assert cnt <= PAD, f"segment {b} has {cnt} points > PAD {PAD}"
        xt = np.zeros((P, F_HALF), dtype=np.float32)
        n1 = min(cnt, F_HALF)
        if n1 > 0:
            xt[0:C, :n1] = features[s : s + n1].T
        if cnt > F_HALF:
            xt[C:P, : cnt - F_HALF] = features[s + F_HALF : e].T
        in_maps.append({
            "xt": xt,
            "invn": np.full((P, 1), 1.0 / max(cnt, 1), dtype=np.float32),
            "gcol": gcol,
            "bcol": bcol,
        })
    return in_maps, bounds


def _assemble(results, bounds):
    out = np.empty((N, C), dtype=np.float32)
    for b in range(BATCH):
        s, e = int(bounds[b]), int(bounds[b + 1])
        cnt = e - s
        if cnt == 0:
            continue
        ot = results[b]["out"]
        n1 = min(cnt, F_HALF)
        out[s : s + n1] = ot[0:C, :n1].T
        if cnt > F_HALF:
            out[s + F_HALF : e] = ot[C:P, : cnt - F_HALF].T
    return out


def run_with_results(features, batch_indices, gamma, beta, **run_kwargs):
    nc = _build_program()
    in_maps, bounds = _prepare(features, batch_indices, gamma, beta)
    res = run_bass_kernel_spmd(nc, in_maps, core_ids=list(range(BATCH)), **run_kwargs)
    return _assemble(res.results, bounds), res


def kernel(features, batch_indices, gamma, beta):
    out, _ = run_with_results(features, batch_indices, gamma, beta)
    return out
